# revision 3
# baseline (speedup 1.0000x reference)
"""Trainium2 Bass kernel for the two-branch (spatial/temporal) attention module.

Computation (full, fp32 reference):
    qkv = x @ Wqkv; q,k,v split -> heads [b,8,n,64]; half = n//2
    all 4096 queries attend to k_t (keys 2048:4096); softmax; out rows
    0:2048 read v rows 0:2048 (spatial), rows 2048:4096 read v rows
    2048:4096 (temporal); concat heads; out @ Wout + b_out.

Sharding (8 cores): core c handles batch c//4 and heads {2*(c%4), 2*(c%4)+1}.
The host sums the 4 partial outputs per batch and adds b_out.

v2 design (per-core):
  - Attention runs in 4 superchunks of 1024 queries, temporal half first so
    compute starts as soon as the second half of x lands.
  - Per (jt, i-half): dots for both heads packed concurrently on the PE
    (row groups 0:64 / 64:128 via tile_position), exp of the [128,1024]
    logits tile on EITHER ScalarE (exact, func=Exp) or the DVE (Schraudolph
    bit-trick: i16 = round(d*a+b), bitcast bf16; ~4% max err on ~40% of
    tiles), alternating so both engines chew the softmax in parallel.
  - AV: lhsT = [v_nat | 1] (65 cols), rhs = E tiles; sums ride along as
    row 64. PSUM: 2x(dots dp 2 banks) + av accumulator 4 banks = 8.
  - Normalization via DMA scatter of the sums row to 128 partitions,
    128-lane reciprocal, stride-0 broadcast back, DVE multiply -> AT bf16.
  - Output projection in bf16 per superchunk, interleaved into the next
    superchunk's PE stream; projections of q/v blocks not needed yet are
    also interleaved to keep the ScalarE/DVE-bound steady state fed.
"""

import sys

sys.path.insert(0, "/opt/trn_rl_repo")

import ml_dtypes
import numpy as np

import concourse.bass as bass
import concourse.mybir as mybir
import concourse.tile as tile
from concourse import bacc
from concourse.bass_utils import run_bass_kernel_spmd
from concourse.masks import make_identity

F32 = mybir.dt.float32
BF16 = mybir.dt.bfloat16
I16 = mybir.dt.int16
AF = mybir.ActivationFunctionType
ALU = mybir.AluOpType

N = 4096
HALF = 2048
DIM = 512
D = 64
SCALE = DIM ** -0.5
LOG2E = 1.4426950408889634
SCH_A = float(SCALE * LOG2E * 128.0)
SCH_B = float((127.0 - 0.0579) * 128.0)

SC_ORDER = [2, 3, 0, 1]          # superchunk order (query blocks of 1024)
DVE_JTS = {1, 4, 7, 10, 13}      # these jt's exp runs on DVE (Schraudolph)


def build_nc():
    nc = bacc.Bacc("TRN2", target_bir_lowering=False, debug=False)

    xT_d = nc.dram_tensor("xT", [DIM, N], BF16, kind="ExternalInput")
    wqkv_d = nc.dram_tensor("Wqkv", [DIM, 384], BF16, kind="ExternalInput")
    wout_d = nc.dram_tensor("Wout", [128, DIM], BF16, kind="ExternalInput")
    outT_d = nc.dram_tensor("outT", [DIM, N], F32, kind="ExternalOutput")

    with tile.TileContext(nc) as tc:
        with (
            tc.tile_pool(name="persist", bufs=1) as persist,
            tc.tile_pool(name="pm", bufs=2, space="PSUM") as pm,   # tag mm: 2x2 banks
            tc.tile_pool(name="pa", bufs=1, space="PSUM") as pa,   # tag av: 1x4 banks
            tc.tile_pool(name="es", bufs=14) as es,
            tc.tile_pool(name="eip", bufs=4) as eip,
            tc.tile_pool(name="sm", bufs=2) as sm,
            tc.tile_pool(name="osb", bufs=4) as osb,
            tc.tile_pool(name="dr", bufs=2, space="DRAM") as dr,
        ):
            kTt = persist.tile([128, HALF], BF16, tag="kTt")
            qTc = [persist.tile([128, 1024], BF16, tag=f"qT{i}", name=f"qT{i}") for i in range(4)]
            vTc = [persist.tile([128, 1024], BF16, tag=f"vT{i}", name=f"vT{i}") for i in range(4)]
            vp = persist.tile([128, 32, 2, 65], BF16, tag="vp")
            wq_s = persist.tile([128, 4, 384], BF16, tag="wq")
            wout_s = persist.tile([128, DIM], BF16, tag="wout")
            ident = persist.tile([128, 128], BF16, tag="ident")
            ATc = [persist.tile([128, 1024], BF16, tag=f"AT{i}", name=f"AT{i}") for i in range(4)]
            xt = [
                [persist.tile([128, HALF], BF16, tag=f"xt{i}_{nh}", name=f"xt{i}_{nh}") for nh in range(2)]
                for i in range(4)
            ]

            # ---------------- loads ------------------------------------------
            nc.sync.dma_start(
                out=wq_s[:, :, :],
                in_=wqkv_d[:, :].rearrange("(t p) c -> p t c", p=128),
            )
            for nh in (1, 0):
                for ct in range(4):
                    nc.sync.dma_start(
                        out=xt[ct][nh][:, :],
                        in_=xT_d[128 * ct : 128 * (ct + 1), 2048 * nh : 2048 * (nh + 1)],
                    )
            nc.sync.dma_start(out=wout_s[:, :], in_=wout_d[:, :])
            make_identity(nc, ident[:, :])
            nc.vector.memset(vp[:, :, :, 64:65], 1.0)

            spin = persist.tile([128, 512], BF16, tag="spin")
            nc.vector.memset(spin[:, :], 1.0)

            def spin_mms(k):
                wps = pm.tile([128, 1024], F32, tag="mm", name="wps")
                for _ in range(k):
                    nc.tensor.matmul(
                        out=wps[:, 0:512], lhsT=spin[:, 0:128], rhs=spin[:, :],
                        start=True, stop=True,
                    )

            spin_mms(16)

            # ---------------- projections ------------------------------------
            copy_flip = [0]

            def alt_copy(out, in_):
                if copy_flip[0] % 2 == 0:
                    nc.vector.tensor_copy(out=out, in_=in_)
                else:
                    nc.scalar.copy(out=out, in_=in_)
                copy_flip[0] += 1

            def proj(dst, wcol0, n0):
                ps = pm.tile([128, 1024], F32, tag="mm", name="ps")
                nh, nb = n0 // HALF, n0 % HALF
                for ct in range(4):
                    for hf in range(2):
                        nc.tensor.matmul(
                            out=ps[:, 512 * hf : 512 * (hf + 1)],
                            lhsT=wq_s[:, ct, wcol0 : wcol0 + 128],
                            rhs=xt[ct][nh][:, nb + 512 * hf : nb + 512 * hf + 512],
                            start=(ct == 0),
                            stop=(ct == 3),
                        )
                alt_copy(dst, ps[:, :])

            def vtrans(jt):
                tp = pm.tile([128, 2, 64], BF16, tag="mm", name="tp")
                nc.tensor.transpose(
                    tp[:, :, :], vTc[jt // 8][:, 128 * (jt % 8) : 128 * (jt % 8 + 1)],
                    ident[:, :],
                )
                nc.vector.tensor_copy(out=vp[:, jt, :, 0:64], in_=tp[:, :, :])

            # ---------------- attention --------------------------------------
            def jt_dots_exp(sc, jt):
                """dots (both heads packed) + exp for both i-halves of one jt."""
                ets = []
                for ih in (0, 1):
                    dp = pm.tile([128, 2, 512], F32, tag="mm", name="dp")
                    for h in (0, 1):
                        nc.tensor.matmul(
                            out=dp[:, h, :],
                            lhsT=kTt[64 * h : 64 * h + 64, 128 * jt : 128 * (jt + 1)],
                            rhs=qTc[sc][64 * h : 64 * h + 64, 512 * ih : 512 * (ih + 1)],
                            start=True, stop=True,
                            tile_position=(64 * h, 0),
                        )
                    if jt in DVE_JTS:
                        ei_t = eip.tile([128, 2, 512], I16, tag="ei", name="ei_t")
                        nc.vector.tensor_scalar(
                            out=ei_t[:, :, :], in0=dp[:, :, :],
                            scalar1=SCH_A, scalar2=SCH_B,
                            op0=ALU.mult, op1=ALU.add,
                        )
                        ets.append(ei_t[:, :, :].bitcast(BF16))
                    else:
                        et = es.tile([128, 2, 512], BF16, tag="es", name="et")
                        nc.scalar.activation(
                            out=et[:, :, :], in_=dp[:, :, :], func=AF.Exp, scale=SCALE
                        )
                        ets.append(et)
                return ets

            def jt_avs(av, voff, jt, ets):
                for h in (0, 1):
                    for ih in (0, 1):
                        nc.tensor.matmul(
                            out=av[:, h, 512 * ih : 512 * (ih + 1)],
                            lhsT=vp[:, voff + jt, h, :],
                            rhs=ets[ih][:, h, :],
                            start=(jt == 0), stop=(jt == 15),
                        )

            def norm(sc, av):
                """stage av, compute AT = av[0:64]/av[64] via scatter-recip-bcast."""
                st = sm.tile([65, 2, 1024], F32, tag="st", name="st")
                nc.vector.tensor_copy(out=st[:, :, :], in_=av[:, :, :])
                sd = dr.tile([2048], F32, tag="sd", name="sd")
                nc.sync.dma_start(out=sd[:], in_=st[64:65, :, :])
                spm = sm.tile([128, 16], F32, tag="spm", name="spm")
                nc.sync.dma_start(
                    out=spm[:, :], in_=sd[:].rearrange("(p t) -> p t", p=128)
                )
                rpm = sm.tile([128, 16], F32, tag="rpm", name="rpm")
                nc.vector.reciprocal(out=rpm[:, :], in_=spm[:, :])
                rd = dr.tile([2048], F32, tag="rd", name="rd")
                nc.sync.dma_start(
                    out=rd[:].rearrange("(p t) -> p t", p=128), in_=rpm[:, :]
                )
                rd_ap = rd[:]
                for h in (0, 1):
                    rb = sm.tile([64, 1024], F32, tag=f"rb{h}", name="rb")
                    rd_b = bass.AP(tensor=rd_ap.tensor, offset=rd_ap.offset + 1024 * h,
                                   ap=[[0, 64], [1, 1024]])
                    nc.sync.dma_start(out=rb[:, :], in_=rd_b)
                    nc.vector.tensor_mul(
                        out=ATc[sc][64 * h : 64 * h + 64, :],
                        in0=st[0:64, h, :], in1=rb[:, :],
                    )

            def outproj_et(sc, et_):
                op = pm.tile([128, 1024], F32, tag="mm", name="op")
                for ih in (0, 1):
                    nc.tensor.matmul(
                        out=op[:, 512 * ih : 512 * (ih + 1)],
                        lhsT=wout_s[:, 128 * et_ : 128 * (et_ + 1)],
                        rhs=ATc[sc][:, 512 * ih : 512 * (ih + 1)],
                        start=True, stop=True,
                    )
                ot = osb.tile([128, 1024], F32, tag="ot", name="ot")
                alt_copy(ot[:, :], op[:, :])
                nc.sync.dma_start(
                    out=outT_d[128 * et_ : 128 * (et_ + 1), 1024 * sc : 1024 * (sc + 1)],
                    in_=ot[:, :],
                )

            # extras emitted at given (sc_idx, jt): lists of thunks
            extras = {
                (0, 0): [lambda: proj(vTc[2][:, :], 256, 2048)],
                (0, 1): [lambda: [vtrans(j) for j in range(16, 20)]],
                (0, 2): [lambda: [vtrans(j) for j in range(20, 24)]],
                (0, 3): [lambda: proj(vTc[3][:, :], 256, 3072)],
                (0, 4): [lambda: [vtrans(j) for j in range(24, 28)]],
                (0, 5): [lambda: [vtrans(j) for j in range(28, 32)]],
                (0, 10): [lambda: proj(qTc[3][:, :], 0, 3072)],
                (1, 2): [lambda: proj(vTc[0][:, :], 256, 0)],
                (1, 4): [lambda: outproj_et(2, 0)],
                (1, 5): [lambda: outproj_et(2, 1)],
                (1, 6): [lambda: outproj_et(2, 2)],
                (1, 7): [lambda: outproj_et(2, 3)],
                (1, 8): [lambda: [vtrans(j) for j in range(0, 4)]],
                (1, 9): [lambda: [vtrans(j) for j in range(4, 8)]],
                (1, 10): [lambda: proj(vTc[1][:, :], 256, 1024)],
                (1, 11): [lambda: [vtrans(j) for j in range(8, 12)]],
                (1, 12): [lambda: [vtrans(j) for j in range(12, 16)]],
                (1, 14): [lambda: proj(qTc[0][:, :], 0, 0)],
                (2, 4): [lambda: outproj_et(3, 0)],
                (2, 5): [lambda: outproj_et(3, 1)],
                (2, 6): [lambda: outproj_et(3, 2)],
                (2, 7): [lambda: outproj_et(3, 3)],
                (2, 10): [lambda: proj(qTc[1][:, :], 0, 1024)],
                (3, 4): [lambda: outproj_et(0, 0)],
                (3, 5): [lambda: outproj_et(0, 1)],
                (3, 6): [lambda: outproj_et(0, 2)],
                (3, 7): [lambda: outproj_et(0, 3)],
            }

            # prologue projections (critical path to first exp)
            proj(kTt[:, 0:1024], 128, HALF)
            proj(kTt[:, 1024:2048], 128, HALF + 1024)
            proj(qTc[2][:, :], 0, 2048)

            for sc_idx, sc in enumerate(SC_ORDER):
                voff = 16 if sc in (2, 3) else 0
                av = pa.tile([65, 2, 1024], F32, tag="av", name="av")
                pending = []
                # AVs held back on sc 0 until the transposes land (jt >= 6)
                av_start = 6 if sc_idx == 0 else 0
                for jt in range(16):
                    ets = jt_dots_exp(sc, jt)
                    pending.append((jt, ets))
                    for thunk in extras.get((sc_idx, jt), []):
                        thunk()
                    if jt >= av_start:
                        while pending:
                            pjt, pets = pending.pop(0)
                            jt_avs(av, voff, pjt, pets)
                while pending:
                    pjt, pets = pending.pop(0)
                    jt_avs(av, voff, pjt, pets)
                norm(sc, av)

            # tail: last superchunk's output projection
            for et_ in range(4):
                outproj_et(1, et_)

    nc.compile()
    return nc


_NC = None


def _get_nc():
    global _NC
    if _NC is None:
        _NC = build_nc()
    return _NC


def shard_inputs(x, Wqkv, Wout):
    bf = ml_dtypes.bfloat16
    ins = []
    for core in range(8):
        b, cp = core // 4, core % 4
        hA = 2 * cp
        xT = np.ascontiguousarray(np.asarray(x[b], np.float32).T).astype(bf)
        wq = Wqkv[:, 64 * hA : 64 * hA + 128]
        wk = Wqkv[:, 512 + 64 * hA : 512 + 64 * hA + 128]
        wv = Wqkv[:, 1024 + 64 * hA : 1024 + 64 * hA + 128]
        wqkv_c = np.concatenate([wq, wk, wv], axis=1).astype(bf)
        wout_c = np.ascontiguousarray(Wout[128 * cp : 128 * cp + 128, :]).astype(bf)
        ins.append({"xT": xT, "Wqkv": wqkv_c, "Wout": wout_c})
    return ins


def run(x, Wqkv, Wout, b_out, trace=False):
    x = np.asarray(x, np.float32)
    Wqkv = np.asarray(Wqkv, np.float32)
    Wout = np.asarray(Wout, np.float32)
    b_out = np.asarray(b_out, np.float32)

    nc = _get_nc()
    ins = shard_inputs(x, Wqkv, Wout)
    res = run_bass_kernel_spmd(nc, ins, list(range(8)), trace=trace)

    out = np.zeros((2, N, DIM), np.float32)
    for core in range(8):
        b = core // 4
        out[b] += res.results[core]["outT"].T
    out += b_out
    return out, res


def kernel(x, Wqkv, Wout, b_out):
    out, _ = run(x, Wqkv, Wout, b_out, trace=False)
    return out


# revision 8
# speedup vs baseline: 1.2564x; 1.2564x over previous
"""Trainium2 Bass kernel for the two-branch (spatial/temporal) attention module.

Computation (full, fp32 reference):
    qkv = x @ Wqkv; q,k,v split -> heads [b,8,n,64]; half = n//2
    all 4096 queries attend to k_t (keys 2048:4096); softmax; out rows
    0:2048 read v rows 0:2048 (spatial), rows 2048:4096 read v rows
    2048:4096 (temporal); concat heads; out @ Wout + b_out.

Sharding (8 cores): core c handles batch c//4 and heads {2*(c%4), 2*(c%4)+1}.
The host sums the 4 partial outputs per batch and adds b_out.

v2 design (per-core):
  - Attention runs in 4 superchunks of 1024 queries, temporal half first so
    compute starts as soon as the second half of x lands.
  - Per (jt, i-half): dots for both heads packed concurrently on the PE
    (row groups 0:64 / 64:128 via tile_position), exp of the [128,1024]
    logits tile on EITHER ScalarE (exact, func=Exp) or the DVE (Schraudolph
    bit-trick: i16 = round(d*a+b), bitcast bf16; ~4% max err on ~40% of
    tiles), alternating so both engines chew the softmax in parallel.
  - AV: lhsT = [v_nat | 1] (65 cols), rhs = E tiles; sums ride along as
    row 64. PSUM: 2x(dots dp 2 banks) + av accumulator 4 banks = 8.
  - Normalization via DMA scatter of the sums row to 128 partitions,
    128-lane reciprocal, stride-0 broadcast back, DVE multiply -> AT bf16.
  - Output projection in bf16 per superchunk, interleaved into the next
    superchunk's PE stream; projections of q/v blocks not needed yet are
    also interleaved to keep the ScalarE/DVE-bound steady state fed.
"""

import sys

sys.path.insert(0, "/opt/trn_rl_repo")

import ml_dtypes
import numpy as np

import concourse.bass as bass
import concourse.mybir as mybir
import concourse.tile as tile
from concourse import bacc
from concourse.bass_utils import run_bass_kernel_spmd
from concourse.masks import make_identity

F32 = mybir.dt.float32
BF16 = mybir.dt.bfloat16
I16 = mybir.dt.int16
AF = mybir.ActivationFunctionType
ALU = mybir.AluOpType

N = 4096
HALF = 2048
DIM = 512
D = 64
SCALE = DIM ** -0.5
LOG2E = 1.4426950408889634
SCH_A = float(SCALE * LOG2E * 128.0)
SCH_B = float((127.0 - 0.0579) * 128.0)

SC_ORDER = [2, 3, 0, 1]          # superchunk order (query blocks of 1024)
DVE_JTS = {1, 4, 7, 9, 12, 14}   # these jt's exp runs on DVE (Schraudolph)


def build_nc():
    nc = bacc.Bacc("TRN2", target_bir_lowering=False, debug=False)

    xT_d = nc.dram_tensor("xT", [DIM, N], BF16, kind="ExternalInput")
    wqkv_d = nc.dram_tensor("Wqkv", [DIM, 384], BF16, kind="ExternalInput")
    wout_d = nc.dram_tensor("Wout", [128, DIM], BF16, kind="ExternalInput")
    outT_d = nc.dram_tensor("outT", [DIM, N], F32, kind="ExternalOutput")

    with tile.TileContext(nc) as tc:
        with (
            tc.tile_pool(name="persist", bufs=1) as persist,
            tc.tile_pool(name="pm", bufs=2, space="PSUM") as pm,   # tag mm: 2x2 banks
            tc.tile_pool(name="pa", bufs=1, space="PSUM") as pa,   # tag av: 1x4 banks
            tc.tile_pool(name="es", bufs=14) as es,
            tc.tile_pool(name="eip", bufs=4) as eip,
            tc.tile_pool(name="sm", bufs=2) as sm,
            tc.tile_pool(name="osb", bufs=4) as osb,
            tc.tile_pool(name="dr", bufs=2, space="DRAM") as dr,
        ):
            kTt = persist.tile([128, HALF], BF16, tag="kTt")
            qTc = [persist.tile([128, 1024], BF16, tag=f"qT{i}", name=f"qT{i}") for i in range(4)]
            vTc = [persist.tile([128, 1024], BF16, tag=f"vT{i}", name=f"vT{i}") for i in range(4)]
            vp = persist.tile([128, 32, 2, 65], BF16, tag="vp")
            wq_s = persist.tile([128, 4, 384], BF16, tag="wq")
            wout_s = persist.tile([128, DIM], BF16, tag="wout")
            ident = persist.tile([128, 128], BF16, tag="ident")
            ATc = [persist.tile([128, 1024], BF16, tag=f"AT{i}", name=f"AT{i}") for i in range(4)]
            xt = [
                [persist.tile([128, HALF], BF16, tag=f"xt{i}_{nh}", name=f"xt{i}_{nh}") for nh in range(2)]
                for i in range(4)
            ]

            # ---------------- loads ------------------------------------------
            nc.sync.dma_start(
                out=wq_s[:, :, :],
                in_=wqkv_d[:, :].rearrange("(t p) c -> p t c", p=128),
            )
            for nh in (1, 0):
                for ct in range(4):
                    nc.sync.dma_start(
                        out=xt[ct][nh][:, :],
                        in_=xT_d[128 * ct : 128 * (ct + 1), 2048 * nh : 2048 * (nh + 1)],
                    )
            nc.sync.dma_start(out=wout_s[:, :], in_=wout_d[:, :])
            make_identity(nc, ident[:, :])
            nc.vector.memset(vp[:, :, :, 64:65], 1.0)

            spin = persist.tile([128, 512], BF16, tag="spin")
            nc.vector.memset(spin[:, :], 1.0)

            def spin_mms(k):
                wps = pm.tile([128, 1024], F32, tag="mm", name="wps")
                for _ in range(k):
                    nc.tensor.matmul(
                        out=wps[:, 0:512], lhsT=spin[:, 0:128], rhs=spin[:, :],
                        start=True, stop=True,
                    )

            spin_mms(16)

            # ---------------- projections ------------------------------------
            def split_copy(dst, src):
                # halve PSUM->SBUF staging across both elementwise engines so
                # neither FIFO stalls the PSUM slot rotation for long
                nc.vector.tensor_copy(out=dst[:, 0:512], in_=src[:, 0:512])
                nc.scalar.copy(out=dst[:, 512:1024], in_=src[:, 512:1024])

            def proj(dst, wcol0, n0):
                ps = pm.tile([128, 1024], F32, tag="mm", name="ps")
                nh, nb = n0 // HALF, n0 % HALF
                for ct in range(4):
                    for hf in range(2):
                        nc.tensor.matmul(
                            out=ps[:, 512 * hf : 512 * (hf + 1)],
                            lhsT=wq_s[:, ct, wcol0 : wcol0 + 128],
                            rhs=xt[ct][nh][:, nb + 512 * hf : nb + 512 * hf + 512],
                            start=(ct == 0),
                            stop=(ct == 3),
                        )
                split_copy(dst, ps[:, :])

            def vtrans4(j0):
                # 4 transposes batched into one PSUM tile, one DVE copy out
                tp = pm.tile([128, 4, 2, 64], BF16, tag="mm", name="tp")
                for k in range(4):
                    j = j0 + k
                    nc.tensor.transpose(
                        tp[:, k, :, :], vTc[j // 8][:, 128 * (j % 8) : 128 * (j % 8 + 1)],
                        ident[:, :],
                    )
                nc.vector.tensor_copy(out=vp[:, j0 : j0 + 4, :, 0:64], in_=tp[:, :, :, :])

            # ---------------- attention --------------------------------------
            def jt_dots_exp(sc, jt):
                """dots (both heads packed) + exp for both i-halves of one jt."""
                ets = []
                for ih in (0, 1):
                    dp = pm.tile([128, 2, 512], F32, tag="mm", name="dp")
                    for h in (0, 1):
                        nc.tensor.matmul(
                            out=dp[:, h, :],
                            lhsT=kTt[64 * h : 64 * h + 64, 128 * jt : 128 * (jt + 1)],
                            rhs=qTc[sc][64 * h : 64 * h + 64, 512 * ih : 512 * (ih + 1)],
                            start=True, stop=True,
                            tile_position=(64 * h, 0),
                        )
                    if jt in DVE_JTS:
                        ei_t = eip.tile([128, 2, 512], I16, tag="ei", name="ei_t")
                        nc.vector.tensor_scalar(
                            out=ei_t[:, :, :], in0=dp[:, :, :],
                            scalar1=SCH_A, scalar2=SCH_B,
                            op0=ALU.mult, op1=ALU.add,
                        )
                        ets.append(ei_t[:, :, :].bitcast(BF16))
                    else:
                        et = es.tile([128, 2, 512], BF16, tag="es", name="et")
                        nc.scalar.activation(
                            out=et[:, :, :], in_=dp[:, :, :], func=AF.Exp, scale=SCALE
                        )
                        ets.append(et)
                return ets

            def jt_avs(av, voff, jt, ets):
                for h in (0, 1):
                    for ih in (0, 1):
                        nc.tensor.matmul(
                            out=av[:, h, 512 * ih : 512 * (ih + 1)],
                            lhsT=vp[:, voff + jt, h, :],
                            rhs=ets[ih][:, h, :],
                            start=(jt == 0), stop=(jt == 15),
                        )

            def norm(sc, av):
                """stage av, compute AT = av[0:64]/av[64] via scatter-recip-bcast."""
                st = sm.tile([65, 2, 1024], F32, tag="st", name="st")
                nc.vector.tensor_copy(out=st[:, :, :], in_=av[:, :, :])
                sd = dr.tile([2048], F32, tag="sd", name="sd")
                nc.sync.dma_start(out=sd[:], in_=st[64:65, :, :])
                spm = sm.tile([128, 16], F32, tag="spm", name="spm")
                nc.sync.dma_start(
                    out=spm[:, :], in_=sd[:].rearrange("(p t) -> p t", p=128)
                )
                rpm = sm.tile([128, 16], F32, tag="rpm", name="rpm")
                nc.vector.reciprocal(out=rpm[:, :], in_=spm[:, :])
                rd = dr.tile([2048], F32, tag="rd", name="rd")
                nc.sync.dma_start(
                    out=rd[:].rearrange("(p t) -> p t", p=128), in_=rpm[:, :]
                )
                rd_ap = rd[:]
                for h in (0, 1):
                    rb = sm.tile([64, 1024], F32, tag=f"rb{h}", name="rb")
                    rd_b = bass.AP(tensor=rd_ap.tensor, offset=rd_ap.offset + 1024 * h,
                                   ap=[[0, 64], [1, 1024]])
                    nc.sync.dma_start(out=rb[:, :], in_=rd_b)
                    nc.vector.tensor_mul(
                        out=ATc[sc][64 * h : 64 * h + 64, :],
                        in0=st[0:64, h, :], in1=rb[:, :],
                    )

            def outproj_et(sc, et_):
                op = pm.tile([128, 1024], F32, tag="mm", name="op")
                for ih in (0, 1):
                    nc.tensor.matmul(
                        out=op[:, 512 * ih : 512 * (ih + 1)],
                        lhsT=wout_s[:, 128 * et_ : 128 * (et_ + 1)],
                        rhs=ATc[sc][:, 512 * ih : 512 * (ih + 1)],
                        start=True, stop=True,
                    )
                ot = osb.tile([128, 1024], F32, tag="ot", name="ot")
                split_copy(ot, op[:, :])
                nc.sync.dma_start(
                    out=outT_d[128 * et_ : 128 * (et_ + 1), 1024 * sc : 1024 * (sc + 1)],
                    in_=ot[:, :],
                )

            # extras emitted at given (sc_idx, jt): lists of thunks
            extras = {
                (0, 0): [lambda: proj(vTc[2][:, :], 256, 2048)],
                (0, 1): [lambda: vtrans4(16)],
                (0, 2): [lambda: vtrans4(20)],
                (0, 3): [lambda: proj(vTc[3][:, :], 256, 3072)],
                (0, 4): [lambda: vtrans4(24)],
                (0, 5): [lambda: vtrans4(28)],
                (0, 10): [lambda: proj(qTc[3][:, :], 0, 3072)],
                (1, 2): [lambda: proj(vTc[0][:, :], 256, 0)],
                (1, 4): [lambda: outproj_et(2, 0)],
                (1, 5): [lambda: outproj_et(2, 1)],
                (1, 6): [lambda: outproj_et(2, 2)],
                (1, 7): [lambda: outproj_et(2, 3)],
                (1, 8): [lambda: vtrans4(0)],
                (1, 9): [lambda: vtrans4(4)],
                (1, 10): [lambda: proj(vTc[1][:, :], 256, 1024)],
                (1, 11): [lambda: vtrans4(8)],
                (1, 12): [lambda: vtrans4(12)],
                (1, 14): [lambda: proj(qTc[0][:, :], 0, 0)],
                (2, 4): [lambda: outproj_et(3, 0)],
                (2, 5): [lambda: outproj_et(3, 1)],
                (2, 6): [lambda: outproj_et(3, 2)],
                (2, 7): [lambda: outproj_et(3, 3)],
                (2, 10): [lambda: proj(qTc[1][:, :], 0, 1024)],
                (3, 4): [lambda: outproj_et(0, 0)],
                (3, 5): [lambda: outproj_et(0, 1)],
                (3, 6): [lambda: outproj_et(0, 2)],
                (3, 7): [lambda: outproj_et(0, 3)],
            }

            # prologue projections (critical path to first exp)
            proj(kTt[:, 0:1024], 128, HALF)
            proj(kTt[:, 1024:2048], 128, HALF + 1024)
            proj(qTc[2][:, :], 0, 2048)

            for sc_idx, sc in enumerate(SC_ORDER):
                voff = 16 if sc in (2, 3) else 0
                av = pa.tile([65, 2, 1024], F32, tag="av", name="av")
                pending = []
                # AVs held back on sc 0 until the transposes land (jt >= 6);
                # afterwards they run one jt behind dots/exp so the PE never
                # waits on the exp of the jt it's AV-ing.
                av_start = 6 if sc_idx == 0 else 1
                for jt in range(16):
                    ets = jt_dots_exp(sc, jt)
                    pending.append((jt, ets))
                    for thunk in extras.get((sc_idx, jt), []):
                        thunk()
                    if jt >= av_start:
                        while len(pending) > 1:
                            pjt, pets = pending.pop(0)
                            jt_avs(av, voff, pjt, pets)
                while pending:
                    pjt, pets = pending.pop(0)
                    jt_avs(av, voff, pjt, pets)
                norm(sc, av)

            # tail: last superchunk's output projection
            for et_ in range(4):
                outproj_et(1, et_)

    nc.compile()
    return nc


_NC = None


def _get_nc():
    global _NC
    if _NC is None:
        _NC = build_nc()
    return _NC


def shard_inputs(x, Wqkv, Wout):
    bf = ml_dtypes.bfloat16
    ins = []
    for core in range(8):
        b, cp = core // 4, core % 4
        hA = 2 * cp
        xT = np.ascontiguousarray(np.asarray(x[b], np.float32).T).astype(bf)
        wq = Wqkv[:, 64 * hA : 64 * hA + 128]
        wk = Wqkv[:, 512 + 64 * hA : 512 + 64 * hA + 128]
        wv = Wqkv[:, 1024 + 64 * hA : 1024 + 64 * hA + 128]
        wqkv_c = np.concatenate([wq, wk, wv], axis=1).astype(bf)
        wout_c = np.ascontiguousarray(Wout[128 * cp : 128 * cp + 128, :]).astype(bf)
        ins.append({"xT": xT, "Wqkv": wqkv_c, "Wout": wout_c})
    return ins


def run(x, Wqkv, Wout, b_out, trace=False):
    x = np.asarray(x, np.float32)
    Wqkv = np.asarray(Wqkv, np.float32)
    Wout = np.asarray(Wout, np.float32)
    b_out = np.asarray(b_out, np.float32)

    nc = _get_nc()
    ins = shard_inputs(x, Wqkv, Wout)
    res = run_bass_kernel_spmd(nc, ins, list(range(8)), trace=trace)

    out = np.zeros((2, N, DIM), np.float32)
    for core in range(8):
        b = core // 4
        out[b] += res.results[core]["outT"].T
    out += b_out
    return out, res


def kernel(x, Wqkv, Wout, b_out):
    out, _ = run(x, Wqkv, Wout, b_out, trace=False)
    return out


# revision 13
# speedup vs baseline: 1.2774x; 1.0167x over previous
"""Trainium2 Bass kernel for the two-branch (spatial/temporal) attention module.

Computation (full, fp32 reference):
    qkv = x @ Wqkv; q,k,v split -> heads [b,8,n,64]; half = n//2
    all 4096 queries attend to k_t (keys 2048:4096); softmax; out rows
    0:2048 read v rows 0:2048 (spatial), rows 2048:4096 read v rows
    2048:4096 (temporal); concat heads; out @ Wout + b_out.

Sharding (8 cores): core c handles batch c//4 and heads {2*(c%4), 2*(c%4)+1}.
The host sums the 4 partial outputs per batch and adds b_out.

v2 design (per-core):
  - Attention runs in 4 superchunks of 1024 queries, temporal half first so
    compute starts as soon as the second half of x lands.
  - Per (jt, i-half): dots for both heads packed concurrently on the PE
    (row groups 0:64 / 64:128 via tile_position), exp of the [128,1024]
    logits tile on EITHER ScalarE (exact, func=Exp) or the DVE (Schraudolph
    bit-trick: i16 = round(d*a+b), bitcast bf16; ~4% max err on ~40% of
    tiles), alternating so both engines chew the softmax in parallel.
  - AV: lhsT = [v_nat | 1] (65 cols), rhs = E tiles; sums ride along as
    row 64. PSUM: 2x(dots dp 2 banks) + av accumulator 4 banks = 8.
  - Normalization via DMA scatter of the sums row to 128 partitions,
    128-lane reciprocal, stride-0 broadcast back, DVE multiply -> AT bf16.
  - Output projection in bf16 per superchunk, interleaved into the next
    superchunk's PE stream; projections of q/v blocks not needed yet are
    also interleaved to keep the ScalarE/DVE-bound steady state fed.
"""

import sys

sys.path.insert(0, "/opt/trn_rl_repo")

import ml_dtypes
import numpy as np

import concourse.bass as bass
import concourse.mybir as mybir
import concourse.tile as tile
from concourse import bacc
from concourse.bass_utils import run_bass_kernel_spmd
from concourse.masks import make_identity

F32 = mybir.dt.float32
BF16 = mybir.dt.bfloat16
I16 = mybir.dt.int16
AF = mybir.ActivationFunctionType
ALU = mybir.AluOpType

N = 4096
HALF = 2048
DIM = 512
D = 64
SCALE = DIM ** -0.5
LOG2E = 1.4426950408889634
SCH_A = float(SCALE * LOG2E * 128.0)
SCH_B = float((127.0 - 0.0579) * 128.0)

SC_ORDER = [2, 3, 0, 1]          # superchunk order (query blocks of 1024)
DVE_JTS = {1, 4, 7, 9, 12, 14}   # these jt's exp runs on DVE (Schraudolph)


def build_nc():
    nc = bacc.Bacc("TRN2", target_bir_lowering=False, debug=False)

    xT_d = nc.dram_tensor("xT", [DIM, N], BF16, kind="ExternalInput")
    wqkv_d = nc.dram_tensor("Wqkv", [DIM, 384], BF16, kind="ExternalInput")
    wout_d = nc.dram_tensor("Wout", [128, DIM], BF16, kind="ExternalInput")
    outT_d = nc.dram_tensor("outT", [DIM, N], F32, kind="ExternalOutput")

    with tile.TileContext(nc) as tc:
        with (
            tc.tile_pool(name="persist", bufs=1) as persist,
            tc.tile_pool(name="pm", bufs=2, space="PSUM") as pm,   # tag mm: 2x2 banks
            tc.tile_pool(name="pa", bufs=1, space="PSUM") as pa,   # tag av: 1x4 banks
            tc.tile_pool(name="es", bufs=16) as es,
            tc.tile_pool(name="eip", bufs=5) as eip,
            tc.tile_pool(name="sm", bufs=2) as sm,
            tc.tile_pool(name="osb", bufs=4) as osb,
            tc.tile_pool(name="dr", bufs=2, space="DRAM") as dr,
        ):
            kTt = persist.tile([128, HALF], BF16, tag="kTt")
            qTc = [persist.tile([128, 1024], BF16, tag=f"qT{i}", name=f"qT{i}") for i in range(4)]
            vTc = [persist.tile([128, 1024], BF16, tag=f"vT{i}", name=f"vT{i}") for i in range(4)]
            vp = persist.tile([128, 32, 2, 65], BF16, tag="vp")
            wq_s = persist.tile([128, 4, 384], BF16, tag="wq")
            wout_s = persist.tile([128, DIM], BF16, tag="wout")
            ident = persist.tile([128, 128], BF16, tag="ident")
            ATc = [persist.tile([128, 1024], BF16, tag=f"AT{i}", name=f"AT{i}") for i in range(4)]
            xt = [
                [persist.tile([128, HALF], BF16, tag=f"xt{i}_{nh}", name=f"xt{i}_{nh}") for nh in range(2)]
                for i in range(4)
            ]

            # ---------------- loads ------------------------------------------
            nc.sync.dma_start(
                out=wq_s[:, :, :],
                in_=wqkv_d[:, :].rearrange("(t p) c -> p t c", p=128),
            )
            for nh in (1, 0):
                for ct in range(4):
                    nc.sync.dma_start(
                        out=xt[ct][nh][:, :],
                        in_=xT_d[128 * ct : 128 * (ct + 1), 2048 * nh : 2048 * (nh + 1)],
                    )
            nc.sync.dma_start(out=wout_s[:, :], in_=wout_d[:, :])
            make_identity(nc, ident[:, :])
            nc.vector.memset(vp[:, :, :, 64:65], 1.0)

            spin = persist.tile([128, 512], BF16, tag="spin")
            nc.vector.memset(spin[:, :], 1.0)

            def spin_mms(k):
                wps = pm.tile([128, 1024], F32, tag="mm", name="wps")
                for _ in range(k):
                    nc.tensor.matmul(
                        out=wps[:, 0:512], lhsT=spin[:, 0:128], rhs=spin[:, :],
                        start=True, stop=True,
                    )

            spin_mms(16)

            # ---------------- projections ------------------------------------
            def split_copy(dst, src):
                # halve PSUM->SBUF staging across both elementwise engines so
                # neither FIFO stalls the PSUM slot rotation for long
                nc.vector.tensor_copy(out=dst[:, 0:512], in_=src[:, 0:512])
                nc.scalar.copy(out=dst[:, 512:1024], in_=src[:, 512:1024])

            def proj(dst, wcol0, n0):
                ps = pm.tile([128, 1024], F32, tag="mm", name="ps")
                nh, nb = n0 // HALF, n0 % HALF
                for ct in range(4):
                    for hf in range(2):
                        nc.tensor.matmul(
                            out=ps[:, 512 * hf : 512 * (hf + 1)],
                            lhsT=wq_s[:, ct, wcol0 : wcol0 + 128],
                            rhs=xt[ct][nh][:, nb + 512 * hf : nb + 512 * hf + 512],
                            start=(ct == 0),
                            stop=(ct == 3),
                        )
                split_copy(dst, ps[:, :])

            def vtrans4(j0):
                # 4 transposes batched into one PSUM tile, one DVE copy out
                tp = pm.tile([128, 4, 2, 64], BF16, tag="mm", name="tp")
                for k in range(4):
                    j = j0 + k
                    nc.tensor.transpose(
                        tp[:, k, :, :], vTc[j // 8][:, 128 * (j % 8) : 128 * (j % 8 + 1)],
                        ident[:, :],
                    )
                nc.vector.tensor_copy(out=vp[:, j0 : j0 + 4, :, 0:64], in_=tp[:, :, :, :])

            # ---------------- attention --------------------------------------
            def jt_dots_exp(sc, jt):
                """dots (both heads packed) + exp for both i-halves of one jt.

                h is the outer loop so each head's kT weights load once per jt
                (2 LDWEIGHTS instead of 4); the second head's pair overlaps the
                first's on the other PE row group.
                """
                dps = [pm.tile([128, 2, 512], F32, tag="mm", name=f"dp{ih}")
                       for ih in (0, 1)]
                for h in (0, 1):
                    for ih in (0, 1):
                        nc.tensor.matmul(
                            out=dps[ih][:, h, :],
                            lhsT=kTt[64 * h : 64 * h + 64, 128 * jt : 128 * (jt + 1)],
                            rhs=qTc[sc][64 * h : 64 * h + 64, 512 * ih : 512 * (ih + 1)],
                            start=True, stop=True,
                            tile_position=(64 * h, 0),
                        )
                ets = []
                for ih in (0, 1):
                    if jt in DVE_JTS:
                        ei_t = eip.tile([128, 2, 512], I16, tag="ei", name="ei_t")
                        nc.vector.tensor_scalar(
                            out=ei_t[:, :, :], in0=dps[ih][:, :, :],
                            scalar1=SCH_A, scalar2=SCH_B,
                            op0=ALU.mult, op1=ALU.add,
                        )
                        ets.append(ei_t[:, :, :].bitcast(BF16))
                    else:
                        et = es.tile([128, 2, 512], BF16, tag="es", name="et")
                        nc.scalar.activation(
                            out=et[:, :, :], in_=dps[ih][:, :, :], func=AF.Exp, scale=SCALE
                        )
                        ets.append(et)
                return ets

            def jt_avs(av, voff, jt, ets):
                for h in (0, 1):
                    for ih in (0, 1):
                        nc.tensor.matmul(
                            out=av[:, h, 512 * ih : 512 * (ih + 1)],
                            lhsT=vp[:, voff + jt, h, :],
                            rhs=ets[ih][:, h, :],
                            start=(jt == 0), stop=(jt == 15),
                        )

            def norm(sc, av):
                """stage av, compute AT = av[0:64]/av[64] via scatter-recip-bcast."""
                st = sm.tile([65, 2, 1024], F32, tag="st", name="st")
                # sums row first so the DMA chain starts early, then the two
                # halves of the staging copy on both elementwise engines
                nc.vector.tensor_copy(out=st[64:65, :, :], in_=av[64:65, :, :])
                spm = sm.tile([128, 16], F32, tag="spm", name="spm")
                nc.sync.dma_start(out=spm[:, :], in_=st[64:65, :, :])
                nc.scalar.copy(out=st[0:64, 0, :], in_=av[0:64, 0, :])
                nc.vector.tensor_copy(out=st[0:64, 1, :], in_=av[0:64, 1, :])
                rpm = sm.tile([128, 16], F32, tag="rpm", name="rpm")
                nc.vector.reciprocal(out=rpm[:, :], in_=spm[:, :])
                rd = dr.tile([2048], F32, tag="rd", name="rd")
                nc.sync.dma_start(
                    out=rd[:].rearrange("(p t) -> p t", p=128), in_=rpm[:, :]
                )
                rd_ap = rd[:]
                for h in (0, 1):
                    rb = sm.tile([64, 1024], F32, tag=f"rb{h}", name="rb")
                    rd_b = bass.AP(tensor=rd_ap.tensor, offset=rd_ap.offset + 1024 * h,
                                   ap=[[0, 64], [1, 1024]])
                    nc.sync.dma_start(out=rb[:, :], in_=rd_b)
                    nc.vector.tensor_mul(
                        out=ATc[sc][64 * h : 64 * h + 64, :],
                        in0=st[0:64, h, :], in1=rb[:, :],
                    )

            def outproj_et(sc, et_):
                op = pm.tile([128, 1024], F32, tag="mm", name="op")
                for ih in (0, 1):
                    nc.tensor.matmul(
                        out=op[:, 512 * ih : 512 * (ih + 1)],
                        lhsT=wout_s[:, 128 * et_ : 128 * (et_ + 1)],
                        rhs=ATc[sc][:, 512 * ih : 512 * (ih + 1)],
                        start=True, stop=True,
                    )
                ot = osb.tile([128, 1024], F32, tag="ot", name="ot")
                split_copy(ot, op[:, :])
                nc.sync.dma_start(
                    out=outT_d[128 * et_ : 128 * (et_ + 1), 1024 * sc : 1024 * (sc + 1)],
                    in_=ot[:, :],
                )

            # extras emitted at given (sc_idx, jt): lists of thunks
            extras = {
                (0, 0): [lambda: proj(kTt[:, 1024:2048], 128, HALF + 1024)],
                (0, 1): [lambda: proj(vTc[2][:, :], 256, 2048)],
                (0, 2): [lambda: vtrans4(16)],
                (0, 3): [lambda: vtrans4(20)],
                (0, 4): [lambda: proj(vTc[3][:, :], 256, 3072)],
                (0, 5): [lambda: vtrans4(24)],
                (0, 6): [lambda: vtrans4(28)],
                (0, 10): [lambda: proj(qTc[3][:, :], 0, 3072)],
                (1, 2): [lambda: proj(vTc[0][:, :], 256, 0)],
                (1, 4): [lambda: outproj_et(2, 0)],
                (1, 6): [lambda: outproj_et(2, 1)],
                (1, 8): [lambda: outproj_et(2, 2)],
                (1, 10): [lambda: outproj_et(2, 3)],
                (1, 9): [lambda: vtrans4(0)],
                (1, 11): [lambda: vtrans4(4)],
                (1, 12): [lambda: proj(vTc[1][:, :], 256, 1024)],
                (1, 13): [lambda: vtrans4(8)],
                (1, 14): [lambda: vtrans4(12), lambda: proj(qTc[0][:, :], 0, 0)],
                (2, 4): [lambda: outproj_et(3, 0)],
                (2, 6): [lambda: outproj_et(3, 1)],
                (2, 8): [lambda: outproj_et(3, 2)],
                (2, 10): [lambda: outproj_et(3, 3)],
                (2, 12): [lambda: proj(qTc[1][:, :], 0, 1024)],
                (3, 4): [lambda: outproj_et(0, 0)],
                (3, 6): [lambda: outproj_et(0, 1)],
                (3, 8): [lambda: outproj_et(0, 2)],
                (3, 10): [lambda: outproj_et(0, 3)],
            }

            # prologue projections (critical path to first exp); kT's second
            # block (keys 1024:2048, first needed at jt=8) moves into sc0
            proj(kTt[:, 0:1024], 128, HALF)
            proj(qTc[2][:, :], 0, 2048)

            for sc_idx, sc in enumerate(SC_ORDER):
                voff = 16 if sc in (2, 3) else 0
                av = pa.tile([65, 2, 1024], F32, tag="av", name="av")
                pending = []
                # AVs held back on sc 0 until the transposes land (jt >= 6);
                # afterwards they run one jt behind dots/exp so the PE never
                # waits on the exp of the jt it's AV-ing.
                av_start = 7 if sc_idx == 0 else 1
                for jt in range(16):
                    ets = jt_dots_exp(sc, jt)
                    pending.append((jt, ets))
                    for thunk in extras.get((sc_idx, jt), []):
                        thunk()
                    if jt >= av_start:
                        while len(pending) > 1:
                            pjt, pets = pending.pop(0)
                            jt_avs(av, voff, pjt, pets)
                while pending:
                    pjt, pets = pending.pop(0)
                    jt_avs(av, voff, pjt, pets)
                norm(sc, av)

            # tail: last superchunk's output projection
            for et_ in range(4):
                outproj_et(1, et_)

    nc.compile()
    return nc


_NC = None


def _get_nc():
    global _NC
    if _NC is None:
        _NC = build_nc()
    return _NC


def shard_inputs(x, Wqkv, Wout):
    bf = ml_dtypes.bfloat16
    ins = []
    for core in range(8):
        b, cp = core // 4, core % 4
        hA = 2 * cp
        xT = np.ascontiguousarray(np.asarray(x[b], np.float32).T).astype(bf)
        wq = Wqkv[:, 64 * hA : 64 * hA + 128]
        wk = Wqkv[:, 512 + 64 * hA : 512 + 64 * hA + 128]
        wv = Wqkv[:, 1024 + 64 * hA : 1024 + 64 * hA + 128]
        wqkv_c = np.concatenate([wq, wk, wv], axis=1).astype(bf)
        wout_c = np.ascontiguousarray(Wout[128 * cp : 128 * cp + 128, :]).astype(bf)
        ins.append({"xT": xT, "Wqkv": wqkv_c, "Wout": wout_c})
    return ins


def run(x, Wqkv, Wout, b_out, trace=False):
    x = np.asarray(x, np.float32)
    Wqkv = np.asarray(Wqkv, np.float32)
    Wout = np.asarray(Wout, np.float32)
    b_out = np.asarray(b_out, np.float32)

    nc = _get_nc()
    ins = shard_inputs(x, Wqkv, Wout)
    res = run_bass_kernel_spmd(nc, ins, list(range(8)), trace=trace)

    out = np.zeros((2, N, DIM), np.float32)
    for core in range(8):
        b = core // 4
        out[b] += res.results[core]["outT"].T
    out += b_out
    return out, res


def kernel(x, Wqkv, Wout, b_out):
    out, _ = run(x, Wqkv, Wout, b_out, trace=False)
    return out


# revision 19
# speedup vs baseline: 1.3691x; 1.0718x over previous
"""Trainium2 Bass kernel for the two-branch (spatial/temporal) attention module.

Computation (full, fp32 reference):
    qkv = x @ Wqkv; q,k,v split -> heads [b,8,n,64]; half = n//2
    all 4096 queries attend to k_t (keys 2048:4096); softmax; out rows
    0:2048 read v rows 0:2048 (spatial), rows 2048:4096 read v rows
    2048:4096 (temporal); concat heads; out @ Wout + b_out.

Sharding (8 cores): core c handles batch c//4 and heads {2*(c%4), 2*(c%4)+1}.
The host sums the 4 partial outputs per batch and adds b_out.

v2 design (per-core):
  - Attention runs in 4 superchunks of 1024 queries, temporal half first so
    compute starts as soon as the second half of x lands.
  - Per (jt, i-half): dots for both heads packed concurrently on the PE
    (row groups 0:64 / 64:128 via tile_position), exp of the [128,1024]
    logits tile on EITHER ScalarE (exact, func=Exp) or the DVE (Schraudolph
    bit-trick: i16 = round(d*a+b), bitcast bf16; ~4% max err on ~40% of
    tiles), alternating so both engines chew the softmax in parallel.
  - AV: lhsT = [v_nat | 1] (65 cols), rhs = E tiles; sums ride along as
    row 64. PSUM: 2x(dots dp 2 banks) + av accumulator 4 banks = 8.
  - Normalization via DMA scatter of the sums row to 128 partitions,
    128-lane reciprocal, stride-0 broadcast back, DVE multiply -> AT bf16.
  - Output projection in bf16 per superchunk, interleaved into the next
    superchunk's PE stream; projections of q/v blocks not needed yet are
    also interleaved to keep the ScalarE/DVE-bound steady state fed.
"""

import sys

sys.path.insert(0, "/opt/trn_rl_repo")

import ml_dtypes
import numpy as np

import concourse.bass as bass
import concourse.mybir as mybir
import concourse.tile as tile
from concourse import bacc
from concourse.bass_utils import run_bass_kernel_spmd
from concourse.masks import make_identity

F32 = mybir.dt.float32
BF16 = mybir.dt.bfloat16
I16 = mybir.dt.int16
AF = mybir.ActivationFunctionType
ALU = mybir.AluOpType

N = 4096
HALF = 2048
DIM = 512
D = 64
SCALE = DIM ** -0.5
LOG2E = 1.4426950408889634
SCH_A = float(SCALE * LOG2E * 128.0)
SCH_B = float((127.0 - 0.0579) * 128.0)

SC_ORDER = [2, 3, 0, 1]          # superchunk order (query blocks of 1024)
DVE_JTS = {1, 4, 7, 9, 12, 14}   # these jt's exp runs on DVE (Schraudolph)


def build_nc():
    nc = bacc.Bacc("TRN2", target_bir_lowering=False, debug=False)

    xT_d = nc.dram_tensor("xT", [DIM, N], BF16, kind="ExternalInput")
    wqkv_d = nc.dram_tensor("Wqkv", [DIM, 384], BF16, kind="ExternalInput")
    wout_d = nc.dram_tensor("Wout", [128, DIM], BF16, kind="ExternalInput")
    outT_d = nc.dram_tensor("outT", [DIM, N], F32, kind="ExternalOutput")

    with tile.TileContext(nc) as tc:
        with (
            tc.tile_pool(name="persist", bufs=1) as persist,
            tc.tile_pool(name="pm", bufs=2, space="PSUM") as pm,   # tag mm: 2x2 banks
            tc.tile_pool(name="pa", bufs=1, space="PSUM") as pa,   # tag av: 1x4 banks
            tc.tile_pool(name="es", bufs=16) as es,
            tc.tile_pool(name="eip", bufs=5) as eip,
            tc.tile_pool(name="sm", bufs=2) as sm,
            tc.tile_pool(name="osb", bufs=4) as osb,
            tc.tile_pool(name="dr", bufs=2, space="DRAM") as dr,
        ):
            kTt = persist.tile([128, HALF], BF16, tag="kTt")
            qTc = [persist.tile([128, 1024], BF16, tag=f"qT{i}", name=f"qT{i}") for i in range(4)]
            vTc = [persist.tile([128, 1024], BF16, tag=f"vT{i}", name=f"vT{i}") for i in range(4)]
            vp = persist.tile([128, 32, 2, 65], BF16, tag="vp")
            wq_s = persist.tile([128, 4, 384], BF16, tag="wq")
            wout_s = persist.tile([128, DIM], BF16, tag="wout")
            ident = persist.tile([128, 128], BF16, tag="ident")
            ATc = [persist.tile([128, 1024], BF16, tag=f"AT{i}", name=f"AT{i}") for i in range(4)]
            xt = [
                [persist.tile([128, HALF], BF16, tag=f"xt{i}_{nh}", name=f"xt{i}_{nh}") for nh in range(2)]
                for i in range(4)
            ]

            # ---------------- loads ------------------------------------------
            nc.sync.dma_start(
                out=wq_s[:, :, :],
                in_=wqkv_d[:, :].rearrange("(t p) c -> p t c", p=128),
            )
            for nh, cb in ((1, 0), (1, 1), (0, 0), (0, 1)):
                for ct in range(4):
                    nc.sync.dma_start(
                        out=xt[ct][nh][:, 1024 * cb : 1024 * (cb + 1)],
                        in_=xT_d[128 * ct : 128 * (ct + 1),
                                 2048 * nh + 1024 * cb : 2048 * nh + 1024 * (cb + 1)],
                    )
            nc.sync.dma_start(out=wout_s[:, :], in_=wout_d[:, :])
            make_identity(nc, ident[:, :])
            nc.vector.memset(vp[:, :, :, 64:65], 1.0)

            spin = persist.tile([128, 512], BF16, tag="spin")
            nc.vector.memset(spin[:, :], 1.0)

            def spin_mms(k):
                wps = pm.tile([128, 1024], F32, tag="mm", name="wps")
                for _ in range(k):
                    nc.tensor.matmul(
                        out=wps[:, 0:512], lhsT=spin[:, 0:128], rhs=spin[:, :],
                        start=True, stop=True,
                    )

            spin_mms(12)

            # ---------------- projections ------------------------------------
            def split_copy(dst, src):
                # halve PSUM->SBUF staging across both elementwise engines so
                # neither FIFO stalls the PSUM slot rotation for long
                nc.vector.tensor_copy(out=dst[:, 0:512], in_=src[:, 0:512])
                nc.scalar.copy(out=dst[:, 512:1024], in_=src[:, 512:1024])

            def proj(dst, wcol0, n0):
                ps = pm.tile([128, 1024], F32, tag="mm", name="ps")
                nh, nb = n0 // HALF, n0 % HALF
                for ct in range(4):
                    for hf in range(2):
                        nc.tensor.matmul(
                            out=ps[:, 512 * hf : 512 * (hf + 1)],
                            lhsT=wq_s[:, ct, wcol0 : wcol0 + 128],
                            rhs=xt[ct][nh][:, nb + 512 * hf : nb + 512 * hf + 512],
                            start=(ct == 0),
                            stop=(ct == 3),
                        )
                split_copy(dst, ps[:, :])

            def vtrans4(j0):
                # 4 transposes batched into one PSUM tile, one DVE copy out
                tp = pm.tile([128, 4, 2, 64], BF16, tag="mm", name="tp")
                for k in range(4):
                    j = j0 + k
                    nc.tensor.transpose(
                        tp[:, k, :, :], vTc[j // 8][:, 128 * (j % 8) : 128 * (j % 8 + 1)],
                        ident[:, :],
                    )
                nc.vector.tensor_copy(out=vp[:, j0 : j0 + 4, :, 0:64], in_=tp[:, :, :, :])

            # ---------------- attention --------------------------------------
            def jt_dots_exp(sc, jt):
                """dots (both heads packed) + exp for both i-halves of one jt.

                h is the outer loop so each head's kT weights load once per jt
                (2 LDWEIGHTS instead of 4); the second head's pair overlaps the
                first's on the other PE row group.
                """
                dps = [pm.tile([128, 2, 512], F32, tag="mm", name=f"dp{ih}")
                       for ih in (0, 1)]
                for h in (0, 1):
                    for ih in (0, 1):
                        nc.tensor.matmul(
                            out=dps[ih][:, h, :],
                            lhsT=kTt[64 * h : 64 * h + 64, 128 * jt : 128 * (jt + 1)],
                            rhs=qTc[sc][64 * h : 64 * h + 64, 512 * ih : 512 * (ih + 1)],
                            start=True, stop=True,
                            tile_position=(64 * h, 0),
                        )
                ets = []
                for ih in (0, 1):
                    if jt in DVE_JTS:
                        ei_t = eip.tile([128, 2, 512], I16, tag="ei", name="ei_t")
                        nc.vector.tensor_scalar(
                            out=ei_t[:, :, :], in0=dps[ih][:, :, :],
                            scalar1=SCH_A, scalar2=SCH_B,
                            op0=ALU.mult, op1=ALU.add,
                        )
                        ets.append(ei_t[:, :, :].bitcast(BF16))
                    else:
                        et = es.tile([128, 2, 512], BF16, tag="es", name="et")
                        nc.scalar.activation(
                            out=et[:, :, :], in_=dps[ih][:, :, :], func=AF.Exp, scale=SCALE
                        )
                        ets.append(et)
                return ets

            def jt_avs(av, voff, jt, ets):
                for h in (0, 1):
                    for ih in (0, 1):
                        nc.tensor.matmul(
                            out=av[:, h, 512 * ih : 512 * (ih + 1)],
                            lhsT=vp[:, voff + jt, h, :],
                            rhs=ets[ih][:, h, :],
                            start=(jt == 0), stop=(jt == 15),
                        )

            def norm(sc, av):
                """stage av, compute AT = av[0:64]/av[64] via scatter-recip-bcast."""
                st = sm.tile([65, 2, 1024], F32, tag="st", name="st")
                # sums row first so the DMA chain starts early, then the two
                # halves of the staging copy on both elementwise engines
                nc.vector.tensor_copy(out=st[64:65, :, :], in_=av[64:65, :, :])
                spm = sm.tile([128, 16], F32, tag="spm", name="spm")
                nc.sync.dma_start(out=spm[:, :], in_=st[64:65, :, :])
                nc.scalar.copy(out=st[0:64, 0, :], in_=av[0:64, 0, :])
                nc.vector.tensor_copy(out=st[0:64, 1, :], in_=av[0:64, 1, :])
                rpm = sm.tile([128, 16], F32, tag="rpm", name="rpm")
                nc.vector.reciprocal(out=rpm[:, :], in_=spm[:, :])
                rd = dr.tile([2048], F32, tag="rd", name="rd")
                nc.sync.dma_start(
                    out=rd[:].rearrange("(p t) -> p t", p=128), in_=rpm[:, :]
                )
                rd_ap = rd[:]
                for h in (0, 1):
                    rb = sm.tile([64, 1024], F32, tag=f"rb{h}", name="rb")
                    rb_src = bass.AP(tensor=rd_ap.tensor,
                                     offset=rd_ap.offset + 1024 * h,
                                     ap=[[0, 64], [1, 1024]])
                    nc.sync.dma_start(out=rb[:, :], in_=rb_src)
                    nc.vector.tensor_mul(
                        out=ATc[sc][64 * h : 64 * h + 64, :],
                        in0=st[0:64, h, :], in1=rb[:, :],
                    )

            def outproj_et(sc, et_):
                op = pm.tile([128, 1024], F32, tag="mm", name="op")
                for ih in (0, 1):
                    nc.tensor.matmul(
                        out=op[:, 512 * ih : 512 * (ih + 1)],
                        lhsT=wout_s[:, 128 * et_ : 128 * (et_ + 1)],
                        rhs=ATc[sc][:, 512 * ih : 512 * (ih + 1)],
                        start=True, stop=True,
                    )
                ot = osb.tile([128, 1024], F32, tag="ot", name="ot")
                split_copy(ot, op[:, :])
                nc.sync.dma_start(
                    out=outT_d[128 * et_ : 128 * (et_ + 1), 1024 * sc : 1024 * (sc + 1)],
                    in_=ot[:, :],
                )

            # extras emitted at given (sc_idx, jt): lists of thunks
            extras = {
                (0, 0): [lambda: proj(kTt[:, 1024:2048], 128, HALF + 1024)],
                (0, 1): [lambda: proj(vTc[2][:, :], 256, 2048)],
                (0, 2): [lambda: vtrans4(16)],
                (0, 3): [lambda: vtrans4(20)],
                (0, 4): [lambda: proj(vTc[3][:, :], 256, 3072)],
                (0, 5): [lambda: vtrans4(24)],
                (0, 6): [lambda: vtrans4(28)],
                (0, 10): [lambda: proj(qTc[3][:, :], 0, 3072)],
                (1, 2): [lambda: proj(vTc[0][:, :], 256, 0)],
                (1, 6): [lambda: outproj_et(2, 0)],
                (1, 8): [lambda: outproj_et(2, 1)],
                (1, 9): [lambda: vtrans4(0)],
                (1, 10): [lambda: outproj_et(2, 2)],
                (1, 11): [lambda: vtrans4(4)],
                (1, 12): [lambda: proj(vTc[1][:, :], 256, 1024)],
                (1, 13): [lambda: outproj_et(2, 3), lambda: vtrans4(8)],
                (1, 14): [lambda: vtrans4(12), lambda: proj(qTc[0][:, :], 0, 0)],
                (2, 6): [lambda: outproj_et(3, 0)],
                (2, 8): [lambda: outproj_et(3, 1)],
                (2, 10): [lambda: outproj_et(3, 2)],
                (2, 13): [lambda: outproj_et(3, 3)],
                (2, 12): [lambda: proj(qTc[1][:, :], 0, 1024)],
                (3, 6): [lambda: outproj_et(0, 0)],
                (3, 8): [lambda: outproj_et(0, 1)],
                (3, 10): [lambda: outproj_et(0, 2)],
                (3, 13): [lambda: outproj_et(0, 3)],
            }

            # prologue projections (critical path to first exp); kT's second
            # block (keys 1024:2048, first needed at jt=8) moves into sc0
            proj(kTt[:, 0:1024], 128, HALF)
            proj(qTc[2][:, :], 0, 2048)

            for sc_idx, sc in enumerate(SC_ORDER):
                voff = 16 if sc in (2, 3) else 0
                av = pa.tile([65, 2, 1024], F32, tag="av", name="av")
                pending = []
                # AVs held back on sc 0 until the transposes land (jt >= 6);
                # afterwards they run one jt behind dots/exp so the PE never
                # waits on the exp of the jt it's AV-ing.
                av_start = 7 if sc_idx == 0 else 1
                for jt in range(16):
                    ets = jt_dots_exp(sc, jt)
                    pending.append((jt, ets))
                    for thunk in extras.get((sc_idx, jt), []):
                        thunk()
                    if jt >= av_start:
                        while len(pending) > 1:
                            pjt, pets = pending.pop(0)
                            jt_avs(av, voff, pjt, pets)
                while pending:
                    pjt, pets = pending.pop(0)
                    jt_avs(av, voff, pjt, pets)
                norm(sc, av)

            # tail: last superchunk's output projection
            for et_ in range(4):
                outproj_et(1, et_)

    nc.compile()
    return nc


_NC = None


def _get_nc():
    global _NC
    if _NC is None:
        _NC = build_nc()
    return _NC


def shard_inputs(x, Wqkv, Wout):
    bf = ml_dtypes.bfloat16
    ins = []
    for core in range(8):
        b, cp = core // 4, core % 4
        hA = 2 * cp
        xT = np.ascontiguousarray(np.asarray(x[b], np.float32).T).astype(bf)
        wq = Wqkv[:, 64 * hA : 64 * hA + 128]
        wk = Wqkv[:, 512 + 64 * hA : 512 + 64 * hA + 128]
        wv = Wqkv[:, 1024 + 64 * hA : 1024 + 64 * hA + 128]
        wqkv_c = np.concatenate([wq, wk, wv], axis=1).astype(bf)
        wout_c = np.ascontiguousarray(Wout[128 * cp : 128 * cp + 128, :]).astype(bf)
        ins.append({"xT": xT, "Wqkv": wqkv_c, "Wout": wout_c})
    return ins


def run(x, Wqkv, Wout, b_out, trace=False):
    x = np.asarray(x, np.float32)
    Wqkv = np.asarray(Wqkv, np.float32)
    Wout = np.asarray(Wout, np.float32)
    b_out = np.asarray(b_out, np.float32)

    nc = _get_nc()
    ins = shard_inputs(x, Wqkv, Wout)
    res = run_bass_kernel_spmd(nc, ins, list(range(8)), trace=trace)

    out = np.zeros((2, N, DIM), np.float32)
    for core in range(8):
        b = core // 4
        out[b] += res.results[core]["outT"].T
    out += b_out
    return out, res


def kernel(x, Wqkv, Wout, b_out):
    out, _ = run(x, Wqkv, Wout, b_out, trace=False)
    return out
